# revision 20
# baseline (speedup 1.0000x reference)
"""Trainium2 Bass kernel for nn_AttentionBlock (B=16, C=512, H=W=32, 8 heads).

Sharding: data-parallel over batch across 8 NeuronCores (2 batch elems/core).
No collectives: each core runs the same NEFF on its own batch slice.

Per-core algorithm (per batch element), all layouts chosen so no transposes
are ever needed:
  x_b is [C=512, S=1024] in natural layout (C on partitions, k-tiled by 128).
  Phase 1 (QKV):
    q/k:   psum[128, S] = [Wq_h | Wk_h]^T @ x  (head h's q,k columns are
           contiguous in w_qkv) -> q_h^T on psum partitions 0-63, k_h^T on
           64-127; copied to SBUF with heads 2a/2a+1 packed on partition
           halves so a pair's score matmuls can co-run on PE row halves.
    v:     out = x^T @ W_v      ->  [S, 8*64] natural layout (S on partitions),
           stored bf16 with a constant ones column appended per head ([.., 65]).
  Phase 2 (attention, per head):
    scores^T = kT.T @ qT  -> psum [128 j, 1024 i]   (K=64; heads of a pair run
               concurrently on PE row-halves 0-63 / 64-127 via tile_position)
    p^T = exp(scores^T * 0.125)  (ScalarE, no max subtraction: |s|<~8, safe)
    [out^T | rowsum] = [v | 1]^T @ p^T  -> psum [65, 1024 i] accum over j tiles
    o^T = out^T * bcast(1/rowsum)  (reciprocal on DVE; broadcast over the 64
          partitions via a K=1 matmul with a ones vector)
  Phase 3: y^T = W_p^T @ o^T + b + x  -> [C, S] natural; DMA out.

Matmuls run as float32r (fp32 data rounded by DVE, single 'High' weight pass,
~4x faster than full fp32) except the P@V stage which is bf16 (p is in [0,1]
scale and v is already truncated by the f32r weight path anyway).
"""

import numpy as np

import concourse.bacc as bacc
import concourse.bass as bass
import concourse.mybir as mybir
import concourse.tile as tile

F32 = mybir.dt.float32
BF16 = mybir.dt.bfloat16
F32R = mybir.dt.float32r

B, C, HW, NH, DK = 16, 512, 1024, 8, 64
NCORES = 8
BPC = B // NCORES          # batch elems per core
P = 128
KT = C // P                # 4 contraction tiles over C
NPAIR = NH // 2            # 4 head pairs
SC = HW // 512             # 2 s-chunks of 512
ST = HW // P               # 8 s-tiles of 128 (j tiles)
SCALE = DK ** -0.5

# 'f32r' | 'f32'  (f32 = exact but ~4x slower matmuls; debug fallback)
MM_DTYPE = "f32r"
MMDT = F32R if MM_DTYPE == "f32r" else F32
# P@V dtype: bf16 (fast, saves SBUF)
PV_BF16 = True


def build_program(with_bias: bool):
    nc = bacc.Bacc(None, target_bir_lowering=False, debug=False)

    x_d = nc.dram_tensor("x", [BPC, C, HW], F32, kind="ExternalInput")
    wqkv_d = nc.dram_tensor("w_qkv", [C, 3 * C], F32, kind="ExternalInput")
    wproj_d = nc.dram_tensor("w_proj", [C, C], F32, kind="ExternalInput")
    if with_bias:
        bqkv_d = nc.dram_tensor("b_qkv", [3 * C], F32, kind="ExternalInput")
        bproj_d = nc.dram_tensor("b_proj", [C], F32, kind="ExternalInput")
    out_d = nc.dram_tensor("out", [BPC, C, HW], F32, kind="ExternalOutput")

    pv_dt = BF16 if PV_BF16 else MMDT

    with tile.TileContext(nc) as tc:
        with tc.tile_pool(name="consts", bufs=1) as consts:
            # Persistent weight buffers (f32r-rounded by DVE as required for
            # fp32r matmul operands).
            wqkv_r = consts.tile([P, KT, NH, 3 * DK], MMDT)
            wv_sb = consts.tile([P, KT, C], MMDT)
            wproj_r = consts.tile([P, KT, C], MMDT)
            ones_f32 = consts.tile([1, P], F32)
            nc.vector.memset(ones_f32, 1.0)
            ones_sb = consts.tile([1, P], MMDT)
            nc.vector.tensor_copy(out=ones_sb, in_=ones_f32)

            if with_bias:
                bq_sb = consts.tile([P, NPAIR], F32)
                bk_sb = consts.tile([P, NPAIR], F32)
                bv_sb = consts.tile([1, C], MMDT)
                bp_sb = consts.tile([P, KT], F32)

            # Staging pool: opened after consts, closed before the main pools
            # so its SBUF is reclaimed for the batch working set.
            with tc.tile_pool(name="wstage", bufs=1) as wstagep:
                wq_st = wstagep.tile([P, KT, 3 * C], F32)
                nc.sync.dma_start(
                    out=wq_st, in_=wqkv_d[:].rearrange("(kt p) n -> p kt n", p=P)
                )
                nc.vector.tensor_copy(
                    out=wqkv_r.rearrange("p kt h t -> p (kt h t)"),
                    in_=wq_st.rearrange("p kt n -> p (kt n)"),
                )
                ws4 = wq_st.rearrange("p kt (h t) -> p kt h t", t=3 * DK)
                for kt in range(KT):
                    nc.vector.tensor_copy(
                        out=wv_sb[:, kt, :].rearrange("p (h t) -> p h t", t=DK),
                        in_=ws4[:, kt, :, 2 * DK :],
                    )
                wp_st = wstagep.tile([P, KT, C], F32)
                nc.sync.dma_start(
                    out=wp_st, in_=wproj_d[:].rearrange("(kt p) n -> p kt n", p=P)
                )
                nc.vector.tensor_copy(
                    out=wproj_r.rearrange("p kt n -> p (kt n)"),
                    in_=wp_st.rearrange("p kt n -> p (kt n)"),
                )
                if with_bias:
                    b3 = bqkv_d[:].rearrange("(h t) -> h t", t=3 * DK)  # [8,192]
                    for m in range(2):
                        # heads m::2 -> partitions m*64.. ; free dim = pair idx
                        nc.sync.dma_start(
                            out=bq_sb[m * DK : (m + 1) * DK, :],
                            in_=b3[m::2, 0:DK].rearrange("a t -> t a"),
                        )
                        nc.sync.dma_start(
                            out=bk_sb[m * DK : (m + 1) * DK, :],
                            in_=b3[m::2, DK : 2 * DK].rearrange("a t -> t a"),
                        )
                    bv_st = wstagep.tile([1, C], F32)
                    nc.sync.dma_start(
                        out=bv_st,
                        in_=b3[:, 2 * DK :].rearrange("h t -> (h t)")[None, :],
                    )
                    nc.vector.tensor_copy(out=bv_sb, in_=bv_st)
                    nc.sync.dma_start(
                        out=bp_sb, in_=bproj_d[:].rearrange("(a p) -> p a", p=P)
                    )

            # Funnel all staging deps through one barrier so the batch-loop
            # DMAs don't inherit a multi-semaphore wait set (HW DMA
            # instructions have very few wait slots).
            tc.strict_bb_all_engine_barrier()

            with (
                tc.tile_pool(name="xp", bufs=1) as xp,
                tc.tile_pool(name="qkt", bufs=1) as qktp,
                tc.tile_pool(name="vp", bufs=1) as vp,
                tc.tile_pool(name="pt", bufs=3) as ptp,
                tc.tile_pool(name="ot", bufs=1) as otp,
                tc.tile_pool(name="stage", bufs=2) as stagep,
                tc.tile_pool(name="rc", bufs=3) as rcp,
                tc.tile_pool(name="psS", bufs=2, space="PSUM") as psS,
                tc.tile_pool(name="psV", bufs=2, space="PSUM") as psV,
                tc.tile_pool(name="psR", bufs=2, space="PSUM") as psR,
            ):
                for b in range(BPC):
                    # ---- load x_b as [P, KT, S]; keep exact f32 for the
                    # residual plus a DVE-rounded f32r copy for matmuls.
                    x_t = xp.tile([P, KT, HW], F32, tag="x", name=f"x{b}")
                    nc.sync.dma_start(
                        out=x_t, in_=x_d[b].rearrange("(kt p) s -> p kt s", p=P)
                    )
                    x_r = xp.tile([P, KT, HW], MMDT, tag="xr", name=f"xr{b}")
                    nc.vector.tensor_copy(
                        out=x_r.rearrange("p kt s -> p (kt s)"),
                        in_=x_t.rearrange("p kt s -> p (kt s)"),
                    )

                    # ---- phase 1: q^T/k^T per head ----
                    # qkT[P, {q,k}, pair, S]: partitions 0-63 head 2a,
                    # 64-127 head 2a+1.
                    qkT = qktp.tile([P, 2, NPAIR, HW], MMDT, tag="qkT",
                                    name=f"qkT{b}")
                    for h in range(NH):
                        a, m = h // 2, h % 2
                        ps = psS.tile([P, 1024], F32, tag="mm", name=f"ps_qk{h}")
                        for sc in range(SC):
                            for kt in range(KT):
                                nc.tensor.matmul(
                                    ps[:, sc * 512 : (sc + 1) * 512],
                                    lhsT=wqkv_r[:, kt, h, 0 : 2 * DK],
                                    rhs=x_r[:, kt, sc * 512 : (sc + 1) * 512],
                                    start=(kt == 0),
                                    stop=(kt == KT - 1),
                                )
                        half = slice(m * DK, (m + 1) * DK)
                        for qk in range(2):  # 0=q (psum 0:64), 1=k (psum 64:128)
                            src = ps[qk * DK : (qk + 1) * DK, :]
                            if with_bias:
                                bsb = bq_sb if qk == 0 else bk_sb
                                nc.vector.tensor_scalar(
                                    out=qkT[half, qk, a, :],
                                    in0=src,
                                    scalar1=bsb[half, a : a + 1],
                                    scalar2=None,
                                    op0=mybir.AluOpType.add,
                                )
                            else:
                                nc.vector.tensor_copy(
                                    out=qkT[half, qk, a, :], in_=src
                                )

                    # ---- phase 1: v natural [P(s), st, head, 65], ones col ----
                    v_sb = vp.tile([P, ST, NH, DK + 1], pv_dt, tag="v",
                                   name=f"v{b}")
                    nc.vector.memset(
                        v_sb.rearrange("p st h t -> p (st h) t")[:, :, DK:], 1.0
                    )
                    for mt2 in range(ST // 2):
                        ps = psS.tile([P, 1024], F32, tag="mm", name=f"ps_v{mt2}")
                        for half_i in range(2):
                            mt = 2 * mt2 + half_i
                            for kt in range(KT):
                                nc.tensor.matmul(
                                    ps[:, half_i * 512 : (half_i + 1) * 512],
                                    lhsT=x_r[:, kt, mt * P : (mt + 1) * P],
                                    rhs=wv_sb[:, kt, :],
                                    start=(kt == 0),
                                    stop=(kt == KT - 1) if not with_bias else False,
                                )
                            if with_bias:
                                # += ones^T @ b_v (adds b_v to every row)
                                nc.tensor.matmul(
                                    ps[:, half_i * 512 : (half_i + 1) * 512],
                                    lhsT=ones_sb,
                                    rhs=bv_sb,
                                    start=False,
                                    stop=True,
                                )
                        for half_i in range(2):
                            mt = 2 * mt2 + half_i
                            nc.vector.tensor_copy(
                                out=v_sb[:, mt, :, 0:DK],
                                in_=ps[:, half_i * 512 : (half_i + 1) * 512]
                                .rearrange("p (h t) -> p h t", h=NH),
                            )

                    # ---- phase 2: attention per head pair ----
                    oT = otp.tile([P, NPAIR, HW], MMDT, tag="oT", name=f"oT{b}")
                    for a in range(NPAIR):
                        pts = [
                            ptp.tile([P, ST, HW], pv_dt, tag="pt",
                                     name=f"pt{a}_{m}")
                            for m in range(2)
                        ]
                        # scores^T + exp; pair co-runs on PE row halves
                        for jt in range(ST):
                            pss = [
                                psS.tile([P, 1024], F32, tag="mm",
                                         name=f"ps_s{m}")
                                for m in range(2)
                            ]
                            for sc in range(SC):
                                for m in range(2):
                                    lo, hi = m * DK, (m + 1) * DK
                                    nc.tensor.matmul(
                                        pss[m][:, sc * 512 : (sc + 1) * 512],
                                        lhsT=qkT[lo:hi, 1, a,
                                                 jt * P : (jt + 1) * P],
                                        rhs=qkT[lo:hi, 0, a,
                                                sc * 512 : (sc + 1) * 512],
                                        start=True,
                                        stop=True,
                                    )
                            for m in range(2):
                                nc.scalar.activation(
                                    out=pts[m][:, jt, :],
                                    in_=pss[m],
                                    func=mybir.ActivationFunctionType.Exp,
                                    scale=SCALE,
                                )

                        # P@V + rowsum, normalize -> oT
                        for m in range(2):
                            h = 2 * a + m
                            for sc in range(SC):
                                pv = psV.tile([DK + 1, 512], F32, tag="pv",
                                              name="pv")
                                for jt in range(ST):
                                    nc.tensor.matmul(
                                        pv,
                                        lhsT=v_sb[:, jt, h, :],
                                        rhs=pts[m][:, jt,
                                                   sc * 512 : (sc + 1) * 512],
                                        start=(jt == 0),
                                        stop=(jt == ST - 1),
                                    )
                                rc = rcp.tile([1, 512], F32, tag="rc", name="rc")
                                nc.vector.reciprocal(rc, pv[DK : DK + 1, :])
                                rcr = rcp.tile([1, 512], MMDT, tag="rcr",
                                               name="rcr")
                                nc.vector.tensor_copy(out=rcr, in_=rc)
                                rb = psR.tile([P, 512], F32, tag="rb", name="rb")
                                nc.tensor.matmul(
                                    rb, lhsT=ones_sb, rhs=rcr,
                                    start=True, stop=True,
                                )
                                osl = oT[m * DK : (m + 1) * DK, a,
                                         sc * 512 : (sc + 1) * 512]
                                nc.vector.tensor_copy(out=osl, in_=pv[0:DK, :])
                                nc.vector.tensor_tensor(
                                    out=osl, in0=osl.bitcast(F32),
                                    in1=rb[0:DK, :],
                                    op=mybir.AluOpType.mult,
                                )

                    # ---- phase 3: proj + bias + residual ----
                    for a in range(KT):
                        ps = psS.tile([P, 1024], F32, tag="mm", name=f"ps_p{a}")
                        for sc in range(SC):
                            for kt in range(KT):
                                nc.tensor.matmul(
                                    ps[:, sc * 512 : (sc + 1) * 512],
                                    lhsT=wproj_r[:, kt, a * P : (a + 1) * P],
                                    rhs=oT[:, kt, sc * 512 : (sc + 1) * 512],
                                    start=(kt == 0),
                                    stop=(kt == KT - 1),
                                )
                        yt = stagep.tile([P, 1024], F32, tag="y", name=f"yt{a}")
                        if with_bias:
                            nc.vector.tensor_scalar(
                                out=yt, in0=ps, scalar1=bp_sb[:, a : a + 1],
                                scalar2=None, op0=mybir.AluOpType.add,
                            )
                            nc.vector.tensor_tensor(
                                out=yt, in0=yt, in1=x_t[:, a, :],
                                op=mybir.AluOpType.add,
                            )
                        else:
                            nc.vector.tensor_tensor(
                                out=yt, in0=ps, in1=x_t[:, a, :],
                                op=mybir.AluOpType.add,
                            )
                        nc.sync.dma_start(
                            out=out_d[b].rearrange("(kt p) s -> p kt s", p=P)
                            [:, a, :],
                            in_=yt,
                        )

    nc.finalize()
    return nc


_CACHE = {}


def _get_program(with_bias: bool):
    if with_bias not in _CACHE:
        _CACHE[with_bias] = build_program(with_bias)
    return _CACHE[with_bias]


def kernel(x, w_qkv, b_qkv, w_proj, b_proj):
    x = np.ascontiguousarray(np.asarray(x, dtype=np.float32)).reshape(B, C, HW)
    w_qkv = np.ascontiguousarray(np.asarray(w_qkv, dtype=np.float32))
    b_qkv = np.ascontiguousarray(np.asarray(b_qkv, dtype=np.float32))
    w_proj = np.ascontiguousarray(np.asarray(w_proj, dtype=np.float32))
    b_proj = np.ascontiguousarray(np.asarray(b_proj, dtype=np.float32))

    with_bias = bool(np.any(b_qkv) or np.any(b_proj))
    nc = _get_program(with_bias)

    in_maps = []
    for i in range(NCORES):
        m = {
            "x": x[i * BPC : (i + 1) * BPC],
            "w_qkv": w_qkv,
            "w_proj": w_proj,
        }
        if with_bias:
            m["b_qkv"] = b_qkv
            m["b_proj"] = b_proj
        in_maps.append(m)

    from concourse.bass_utils import run_bass_kernel_spmd

    res = run_bass_kernel_spmd(nc, in_maps, core_ids=list(range(NCORES)))
    out = np.concatenate([r["out"] for r in res.results], axis=0)
    return out.reshape(B, C, 32, 32)


# revision 23
# speedup vs baseline: 1.0938x; 1.0938x over previous
"""Trainium2 Bass kernel for nn_AttentionBlock (B=16, C=512, H=W=32, 8 heads).

Sharding: data-parallel over batch across 8 NeuronCores (2 batch elems/core).
No collectives: each core runs the same NEFF on its own batch slice.

Per-core algorithm (per batch element), all layouts chosen so no transposes
are ever needed:
  x_b is [C=512, S=1024] in natural layout (C on partitions, k-tiled by 128).
  Phase 1 (QKV):
    q/k:   psum[128, S] = [Wq_h | Wk_h]^T @ x  (head h's q,k columns are
           contiguous in w_qkv) -> q_h^T on psum partitions 0-63, k_h^T on
           64-127; copied to SBUF with heads 2a/2a+1 packed on partition
           halves so a pair's score matmuls can co-run on PE row halves.
    v:     out = x^T @ W_v      ->  [S, 8*64] natural layout (S on partitions),
           stored bf16 with a constant ones column appended per head ([.., 65]).
  Phase 2 (attention, per head):
    scores^T = kT.T @ qT  -> psum [128 j, 1024 i]   (K=64; heads of a pair run
               concurrently on PE row-halves 0-63 / 64-127 via tile_position)
    p^T = exp(scores^T * 0.125)  (ScalarE, no max subtraction: |s|<~8, safe)
    [out^T | rowsum] = [v | 1]^T @ p^T  -> psum [65, 1024 i] accum over j tiles
    o^T = out^T * bcast(1/rowsum)  (reciprocal on DVE; broadcast over the 64
          partitions via a K=1 matmul with a ones vector)
  Phase 3: y^T = W_p^T @ o^T + b + x  -> [C, S] natural; DMA out.

Matmuls run as float32r (fp32 data rounded by DVE, single 'High' weight pass,
~4x faster than full fp32) except the P@V stage which is bf16 (p is in [0,1]
scale and v is already truncated by the f32r weight path anyway).
"""

import numpy as np

import concourse.bacc as bacc
import concourse.bass as bass
import concourse.mybir as mybir
import concourse.tile as tile

F32 = mybir.dt.float32
BF16 = mybir.dt.bfloat16
F32R = mybir.dt.float32r

B, C, HW, NH, DK = 16, 512, 1024, 8, 64
NCORES = 8
BPC = B // NCORES          # batch elems per core
P = 128
KT = C // P                # 4 contraction tiles over C
NPAIR = NH // 2            # 4 head pairs
SC = HW // 512             # 2 s-chunks of 512
ST = HW // P               # 8 s-tiles of 128 (j tiles)
SCALE = DK ** -0.5

# 'f32r' | 'f32'  (f32 = exact but ~4x slower matmuls; debug fallback)
MM_DTYPE = "f32r"
MMDT = F32R if MM_DTYPE == "f32r" else F32
# P@V dtype: bf16 (fast, saves SBUF)
PV_BF16 = True


def build_program(with_bias: bool):
    nc = bacc.Bacc(None, target_bir_lowering=False, debug=False)

    x_d = nc.dram_tensor("x", [BPC, C, HW], F32, kind="ExternalInput")
    wqkv_d = nc.dram_tensor("w_qkv", [C, 3 * C], F32, kind="ExternalInput")
    wproj_d = nc.dram_tensor("w_proj", [C, C], F32, kind="ExternalInput")
    if with_bias:
        bqkv_d = nc.dram_tensor("b_qkv", [3 * C], F32, kind="ExternalInput")
        bproj_d = nc.dram_tensor("b_proj", [C], F32, kind="ExternalInput")
    out_d = nc.dram_tensor("out", [BPC, C, HW], F32, kind="ExternalOutput")

    pv_dt = BF16 if PV_BF16 else MMDT

    with tile.TileContext(nc) as tc:
        with tc.tile_pool(name="consts", bufs=1) as consts:
            # Persistent weight buffers (f32r-rounded by DVE as required for
            # fp32r matmul operands).
            wqkv_r = consts.tile([P, KT, NH, 3 * DK], MMDT)
            wv_sb = consts.tile([P, KT, C], MMDT)
            wproj_r = consts.tile([P, KT, C], MMDT)
            ones_f32 = consts.tile([1, P], F32)
            nc.vector.memset(ones_f32, 1.0)
            ones_sb = consts.tile([1, P], MMDT)
            nc.vector.tensor_copy(out=ones_sb, in_=ones_f32)

            if with_bias:
                bq_sb = consts.tile([P, NPAIR], F32)
                bk_sb = consts.tile([P, NPAIR], F32)
                bv_sb = consts.tile([1, C], MMDT)
                bp_sb = consts.tile([P, KT], F32)

            # Staging pool: opened after consts, closed before the main pools
            # so its SBUF is reclaimed for the batch working set.
            with tc.tile_pool(name="wstage", bufs=1) as wstagep:
                wq_st = wstagep.tile([P, KT, 3 * C], F32)
                nc.sync.dma_start(
                    out=wq_st, in_=wqkv_d[:].rearrange("(kt p) n -> p kt n", p=P)
                )
                nc.vector.tensor_copy(
                    out=wqkv_r.rearrange("p kt h t -> p (kt h t)"),
                    in_=wq_st.rearrange("p kt n -> p (kt n)"),
                )
                ws4 = wq_st.rearrange("p kt (h t) -> p kt h t", t=3 * DK)
                for kt in range(KT):
                    nc.vector.tensor_copy(
                        out=wv_sb[:, kt, :].rearrange("p (h t) -> p h t", t=DK),
                        in_=ws4[:, kt, :, 2 * DK :],
                    )
                wp_st = wstagep.tile([P, KT, C], F32)
                nc.sync.dma_start(
                    out=wp_st, in_=wproj_d[:].rearrange("(kt p) n -> p kt n", p=P)
                )
                nc.vector.tensor_copy(
                    out=wproj_r.rearrange("p kt n -> p (kt n)"),
                    in_=wp_st.rearrange("p kt n -> p (kt n)"),
                )
                if with_bias:
                    b3 = bqkv_d[:].rearrange("(h t) -> h t", t=3 * DK)  # [8,192]
                    for m in range(2):
                        # heads m::2 -> partitions m*64.. ; free dim = pair idx
                        nc.sync.dma_start(
                            out=bq_sb[m * DK : (m + 1) * DK, :],
                            in_=b3[m::2, 0:DK].rearrange("a t -> t a"),
                        )
                        nc.sync.dma_start(
                            out=bk_sb[m * DK : (m + 1) * DK, :],
                            in_=b3[m::2, DK : 2 * DK].rearrange("a t -> t a"),
                        )
                    bv_st = wstagep.tile([1, C], F32)
                    nc.sync.dma_start(
                        out=bv_st,
                        in_=b3[:, 2 * DK :].rearrange("h t -> (h t)")[None, :],
                    )
                    nc.vector.tensor_copy(out=bv_sb, in_=bv_st)
                    nc.sync.dma_start(
                        out=bp_sb, in_=bproj_d[:].rearrange("(a p) -> p a", p=P)
                    )

            # Funnel all staging deps through one barrier so the batch-loop
            # DMAs don't inherit a multi-semaphore wait set (HW DMA
            # instructions have very few wait slots).
            tc.strict_bb_all_engine_barrier()

            with (
                tc.tile_pool(name="xp", bufs=1) as xp,
                tc.tile_pool(name="qkt", bufs=1) as qktp,
                tc.tile_pool(name="vp", bufs=1) as vp,
                tc.tile_pool(name="pt", bufs=3) as ptp,
                tc.tile_pool(name="ot", bufs=1) as otp,
                tc.tile_pool(name="stage", bufs=2) as stagep,
                tc.tile_pool(name="rc", bufs=3) as rcp,
                tc.tile_pool(name="rcb", bufs=3) as rcbp,
                tc.tile_pool(name="psS", bufs=2, space="PSUM") as psS,
                tc.tile_pool(name="psV", bufs=4, space="PSUM") as psV,
            ):
                for b in range(BPC):
                    # ---- load x_b as [P, KT, S]; keep exact f32 for the
                    # residual plus a DVE-rounded f32r copy for matmuls.
                    x_t = xp.tile([P, KT, HW], F32, tag="x", name=f"x{b}")
                    nc.sync.dma_start(
                        out=x_t, in_=x_d[b].rearrange("(kt p) s -> p kt s", p=P)
                    )
                    x_r = xp.tile([P, KT, HW], MMDT, tag="xr", name=f"xr{b}")
                    nc.vector.tensor_copy(
                        out=x_r.rearrange("p kt s -> p (kt s)"),
                        in_=x_t.rearrange("p kt s -> p (kt s)"),
                    )

                    # ---- phase 1: q^T/k^T per head ----
                    # qkT[P, {q,k}, pair, S]: partitions 0-63 head 2a,
                    # 64-127 head 2a+1.
                    qkT = qktp.tile([P, 2, NPAIR, HW], MMDT, tag="qkT",
                                    name=f"qkT{b}")
                    for h in range(NH):
                        a, m = h // 2, h % 2
                        ps = psS.tile([P, 1024], F32, tag="mm", name=f"ps_qk{h}")
                        for sc in range(SC):
                            for kt in range(KT):
                                nc.tensor.matmul(
                                    ps[:, sc * 512 : (sc + 1) * 512],
                                    lhsT=wqkv_r[:, kt, h, 0 : 2 * DK],
                                    rhs=x_r[:, kt, sc * 512 : (sc + 1) * 512],
                                    start=(kt == 0),
                                    stop=(kt == KT - 1),
                                )
                        half = slice(m * DK, (m + 1) * DK)
                        for qk in range(2):  # 0=q (psum 0:64), 1=k (psum 64:128)
                            src = ps[qk * DK : (qk + 1) * DK, :]
                            if with_bias:
                                bsb = bq_sb if qk == 0 else bk_sb
                                nc.vector.tensor_scalar(
                                    out=qkT[half, qk, a, :],
                                    in0=src,
                                    scalar1=bsb[half, a : a + 1],
                                    scalar2=None,
                                    op0=mybir.AluOpType.add,
                                )
                            else:
                                nc.vector.tensor_copy(
                                    out=qkT[half, qk, a, :], in_=src
                                )

                    # ---- phase 1: v natural [P(s), st, head, 65], ones col ----
                    v_sb = vp.tile([P, ST, NH, DK + 1], pv_dt, tag="v",
                                   name=f"v{b}")
                    nc.vector.memset(
                        v_sb.rearrange("p st h t -> p (st h) t")[:, :, DK:], 1.0
                    )
                    for mt2 in range(ST // 2):
                        ps = psS.tile([P, 1024], F32, tag="mm", name=f"ps_v{mt2}")
                        for half_i in range(2):
                            mt = 2 * mt2 + half_i
                            for kt in range(KT):
                                nc.tensor.matmul(
                                    ps[:, half_i * 512 : (half_i + 1) * 512],
                                    lhsT=x_r[:, kt, mt * P : (mt + 1) * P],
                                    rhs=wv_sb[:, kt, :],
                                    start=(kt == 0),
                                    stop=(kt == KT - 1) if not with_bias else False,
                                )
                            if with_bias:
                                # += ones^T @ b_v (adds b_v to every row)
                                nc.tensor.matmul(
                                    ps[:, half_i * 512 : (half_i + 1) * 512],
                                    lhsT=ones_sb,
                                    rhs=bv_sb,
                                    start=False,
                                    stop=True,
                                )
                        for half_i in range(2):
                            mt = 2 * mt2 + half_i
                            nc.vector.tensor_copy(
                                out=v_sb[:, mt, :, 0:DK],
                                in_=ps[:, half_i * 512 : (half_i + 1) * 512]
                                .rearrange("p (h t) -> p h t", h=NH),
                            )

                    # ---- phase 2: attention per head pair ----
                    oT = otp.tile([P, NPAIR, HW], MMDT, tag="oT", name=f"oT{b}")
                    for a in range(NPAIR):
                        pts = [
                            ptp.tile([P, ST, HW], pv_dt, tag="pt",
                                     name=f"pt{a}_{m}")
                            for m in range(2)
                        ]
                        # scores^T + exp; pair co-runs on PE row halves
                        for jt in range(ST):
                            pss = [
                                psS.tile([P, 1024], F32, tag="mm",
                                         name=f"ps_s{m}")
                                for m in range(2)
                            ]
                            for sc in range(SC):
                                for m in range(2):
                                    lo, hi = m * DK, (m + 1) * DK
                                    nc.tensor.matmul(
                                        pss[m][:, sc * 512 : (sc + 1) * 512],
                                        lhsT=qkT[lo:hi, 1, a,
                                                 jt * P : (jt + 1) * P],
                                        rhs=qkT[lo:hi, 0, a,
                                                sc * 512 : (sc + 1) * 512],
                                        start=True,
                                        stop=True,
                                    )
                            for m in range(2):
                                nc.scalar.activation(
                                    out=pts[m][:, jt, :],
                                    in_=pss[m],
                                    func=mybir.ActivationFunctionType.Exp,
                                    scale=SCALE,
                                )

                        # P@V + rowsum, normalize -> oT
                        for m in range(2):
                            h = 2 * a + m
                            for sc in range(SC):
                                pv = psV.tile([DK + 1, 512], F32, tag="pv",
                                              name="pv")
                                for jt in range(ST):
                                    nc.tensor.matmul(
                                        pv,
                                        lhsT=v_sb[:, jt, h, :],
                                        rhs=pts[m][:, jt,
                                                   sc * 512 : (sc + 1) * 512],
                                        start=(jt == 0),
                                        stop=(jt == ST - 1),
                                    )
                                # 1/rowsum: fast approx (rowsum >= ~1, no edge
                                # cases; 51 ULP is far below our error budget)
                                # then broadcast over 64 partitions on GpSimd.
                                # (custom-DVE ops misread PSUM/offset inputs;
                                # bounce the rowsum to SBUF partition 0 first)
                                rs = rcp.tile([1, 512], F32, tag="rs", name="rs")
                                nc.vector.tensor_copy(
                                    out=rs, in_=pv[DK : DK + 1, :]
                                )
                                rc = rcp.tile([1, 512], F32, tag="rc", name="rc")
                                nc.vector.reciprocal_approx_fast(out=rc, in_=rs)
                                rcb = rcbp.tile([DK, 512], F32, tag="rcb",
                                                name="rcb")
                                nc.gpsimd.partition_broadcast(rcb, rc)
                                osl = oT[m * DK : (m + 1) * DK, a,
                                         sc * 512 : (sc + 1) * 512]
                                nc.vector.tensor_tensor(
                                    out=osl, in0=pv[0:DK, :], in1=rcb,
                                    op=mybir.AluOpType.mult,
                                )

                    # ---- phase 3: proj + bias + residual ----
                    for a in range(KT):
                        ps = psS.tile([P, 1024], F32, tag="mm", name=f"ps_p{a}")
                        for sc in range(SC):
                            for kt in range(KT):
                                nc.tensor.matmul(
                                    ps[:, sc * 512 : (sc + 1) * 512],
                                    lhsT=wproj_r[:, kt, a * P : (a + 1) * P],
                                    rhs=oT[:, kt, sc * 512 : (sc + 1) * 512],
                                    start=(kt == 0),
                                    stop=(kt == KT - 1),
                                )
                        yt = stagep.tile([P, 1024], F32, tag="y", name=f"yt{a}")
                        if with_bias:
                            nc.vector.tensor_scalar(
                                out=yt, in0=ps, scalar1=bp_sb[:, a : a + 1],
                                scalar2=None, op0=mybir.AluOpType.add,
                            )
                            nc.vector.tensor_tensor(
                                out=yt, in0=yt, in1=x_t[:, a, :],
                                op=mybir.AluOpType.add,
                            )
                        else:
                            nc.vector.tensor_tensor(
                                out=yt, in0=ps, in1=x_t[:, a, :],
                                op=mybir.AluOpType.add,
                            )
                        nc.sync.dma_start(
                            out=out_d[b].rearrange("(kt p) s -> p kt s", p=P)
                            [:, a, :],
                            in_=yt,
                        )

    nc.finalize()
    return nc


_CACHE = {}


def _get_program(with_bias: bool):
    if with_bias not in _CACHE:
        _CACHE[with_bias] = build_program(with_bias)
    return _CACHE[with_bias]


def kernel(x, w_qkv, b_qkv, w_proj, b_proj):
    x = np.ascontiguousarray(np.asarray(x, dtype=np.float32)).reshape(B, C, HW)
    w_qkv = np.ascontiguousarray(np.asarray(w_qkv, dtype=np.float32))
    b_qkv = np.ascontiguousarray(np.asarray(b_qkv, dtype=np.float32))
    w_proj = np.ascontiguousarray(np.asarray(w_proj, dtype=np.float32))
    b_proj = np.ascontiguousarray(np.asarray(b_proj, dtype=np.float32))

    with_bias = bool(np.any(b_qkv) or np.any(b_proj))
    nc = _get_program(with_bias)

    in_maps = []
    for i in range(NCORES):
        m = {
            "x": x[i * BPC : (i + 1) * BPC],
            "w_qkv": w_qkv,
            "w_proj": w_proj,
        }
        if with_bias:
            m["b_qkv"] = b_qkv
            m["b_proj"] = b_proj
        in_maps.append(m)

    from concourse.bass_utils import run_bass_kernel_spmd

    res = run_bass_kernel_spmd(nc, in_maps, core_ids=list(range(NCORES)))
    out = np.concatenate([r["out"] for r in res.results], axis=0)
    return out.reshape(B, C, 32, 32)


# revision 25
# speedup vs baseline: 1.3683x; 1.2510x over previous
"""Trainium2 Bass kernel for nn_AttentionBlock (B=16, C=512, H=W=32, 8 heads).

Sharding: data-parallel over batch across 8 NeuronCores (2 batch elems/core).
No collectives: each core runs the same NEFF on its own batch slice.

Per-core algorithm (per batch element), all layouts chosen so no transposes
are ever needed:
  x_b is [C=512, S=1024] in natural layout (C on partitions, k-tiled by 128).
  Phase 1 (QKV):
    q/k:   psum[128, S] = [Wq_h | Wk_h]^T @ x  (head h's q,k columns are
           contiguous in w_qkv) -> q_h^T on psum partitions 0-63, k_h^T on
           64-127; copied to SBUF with heads 2a/2a+1 packed on partition
           halves so a pair's score matmuls can co-run on PE row halves.
    v:     out = x^T @ W_v      ->  [S, 8*64] natural layout (S on partitions),
           stored bf16 with a constant ones column appended per head ([.., 65]).
  Phase 2 (attention, per head):
    scores^T = kT.T @ qT  -> psum [128 j, 1024 i]   (K=64; heads of a pair run
               concurrently on PE row-halves 0-63 / 64-127 via tile_position)
    p^T = exp(scores^T * 0.125)  (ScalarE, no max subtraction: |s|<~8, safe)
    [out^T | rowsum] = [v | 1]^T @ p^T  -> psum [65, 1024 i] accum over j tiles
    o^T = out^T * bcast(1/rowsum)  (reciprocal on DVE; broadcast over the 64
          partitions via a K=1 matmul with a ones vector)
  Phase 3: y^T = W_p^T @ o^T + b + x  -> [C, S] natural; DMA out.

Matmuls run as float32r (fp32 data rounded by DVE, single 'High' weight pass,
~4x faster than full fp32) except the P@V stage which is bf16 (p is in [0,1]
scale and v is already truncated by the f32r weight path anyway).
"""

import numpy as np

import concourse.bacc as bacc
import concourse.bass as bass
import concourse.mybir as mybir
import concourse.tile as tile

F32 = mybir.dt.float32
BF16 = mybir.dt.bfloat16
F32R = mybir.dt.float32r

B, C, HW, NH, DK = 16, 512, 1024, 8, 64
NCORES = 8
BPC = B // NCORES          # batch elems per core
P = 128
KT = C // P                # 4 contraction tiles over C
NPAIR = NH // 2            # 4 head pairs
SC = HW // 512             # 2 s-chunks of 512
ST = HW // P               # 8 s-tiles of 128 (j tiles)
SCALE = DK ** -0.5

# 'f32r' | 'f32'  (f32 = exact but ~4x slower matmuls; debug fallback)
MM_DTYPE = "f32r"
# P@V dtype: bf16 (fast, saves SBUF)
PV_BF16 = True


def build_program(with_bias: bool, mm_dtype: str = MM_DTYPE):
    MMDT = {"f32r": F32R, "bf16": BF16, "f32": F32}[mm_dtype]
    nc = bacc.Bacc(None, target_bir_lowering=False, debug=False)

    x_d = nc.dram_tensor("x", [BPC, C, HW], F32, kind="ExternalInput")
    wqkv_d = nc.dram_tensor("w_qkv", [C, 3 * C], F32, kind="ExternalInput")
    wproj_d = nc.dram_tensor("w_proj", [C, C], F32, kind="ExternalInput")
    if with_bias:
        bqkv_d = nc.dram_tensor("b_qkv", [3 * C], F32, kind="ExternalInput")
        bproj_d = nc.dram_tensor("b_proj", [C], F32, kind="ExternalInput")
    out_d = nc.dram_tensor("out", [BPC, C, HW], F32, kind="ExternalOutput")

    pv_dt = BF16 if PV_BF16 else MMDT

    with tile.TileContext(nc) as tc:
        with tc.tile_pool(name="consts", bufs=1) as consts:
            # Persistent weight buffers (f32r-rounded by DVE as required for
            # fp32r matmul operands).
            wqkv_r = consts.tile([P, KT, NH, 3 * DK], MMDT)
            wv_sb = consts.tile([P, KT, C], MMDT)
            wproj_r = consts.tile([P, KT, C], MMDT)
            ones_f32 = consts.tile([1, P], F32)
            nc.vector.memset(ones_f32, 1.0)
            ones_sb = consts.tile([1, P], MMDT)
            nc.vector.tensor_copy(out=ones_sb, in_=ones_f32)

            if with_bias:
                bq_sb = consts.tile([P, NPAIR], F32)
                bk_sb = consts.tile([P, NPAIR], F32)
                bv_sb = consts.tile([1, C], MMDT)
                bp_sb = consts.tile([P, KT], F32)

            # Staging pool: opened after consts, closed before the main pools
            # so its SBUF is reclaimed for the batch working set.
            with tc.tile_pool(name="wstage", bufs=1) as wstagep:
                wq_st = wstagep.tile([P, KT, 3 * C], F32)
                nc.sync.dma_start(
                    out=wq_st, in_=wqkv_d[:].rearrange("(kt p) n -> p kt n", p=P)
                )
                nc.vector.tensor_copy(
                    out=wqkv_r.rearrange("p kt h t -> p (kt h t)"),
                    in_=wq_st.rearrange("p kt n -> p (kt n)"),
                )
                ws4 = wq_st.rearrange("p kt (h t) -> p kt h t", t=3 * DK)
                for kt in range(KT):
                    nc.vector.tensor_copy(
                        out=wv_sb[:, kt, :].rearrange("p (h t) -> p h t", t=DK),
                        in_=ws4[:, kt, :, 2 * DK :],
                    )
                wp_st = wstagep.tile([P, KT, C], F32)
                nc.sync.dma_start(
                    out=wp_st, in_=wproj_d[:].rearrange("(kt p) n -> p kt n", p=P)
                )
                nc.vector.tensor_copy(
                    out=wproj_r.rearrange("p kt n -> p (kt n)"),
                    in_=wp_st.rearrange("p kt n -> p (kt n)"),
                )
                if with_bias:
                    b3 = bqkv_d[:].rearrange("(h t) -> h t", t=3 * DK)  # [8,192]
                    for m in range(2):
                        # heads m::2 -> partitions m*64.. ; free dim = pair idx
                        nc.sync.dma_start(
                            out=bq_sb[m * DK : (m + 1) * DK, :],
                            in_=b3[m::2, 0:DK].rearrange("a t -> t a"),
                        )
                        nc.sync.dma_start(
                            out=bk_sb[m * DK : (m + 1) * DK, :],
                            in_=b3[m::2, DK : 2 * DK].rearrange("a t -> t a"),
                        )
                    bv_st = wstagep.tile([1, C], F32)
                    nc.sync.dma_start(
                        out=bv_st,
                        in_=b3[:, 2 * DK :].rearrange("h t -> (h t)")[None, :],
                    )
                    nc.vector.tensor_copy(out=bv_sb, in_=bv_st)
                    nc.sync.dma_start(
                        out=bp_sb, in_=bproj_d[:].rearrange("(a p) -> p a", p=P)
                    )

            # Funnel all staging deps through one barrier so the batch-loop
            # DMAs don't inherit a multi-semaphore wait set (HW DMA
            # instructions have very few wait slots).
            tc.strict_bb_all_engine_barrier()

            with (
                tc.tile_pool(name="xp", bufs=1) as xp,
                tc.tile_pool(name="qkt", bufs=1) as qktp,
                tc.tile_pool(name="vp", bufs=1) as vp,
                tc.tile_pool(name="pt", bufs=3) as ptp,
                tc.tile_pool(name="ot", bufs=1) as otp,
                tc.tile_pool(name="stage", bufs=2) as stagep,
                tc.tile_pool(name="rc", bufs=3) as rcp,
                tc.tile_pool(name="rcb", bufs=3) as rcbp,
                tc.tile_pool(name="psS", bufs=2, space="PSUM") as psS,
                tc.tile_pool(name="psV", bufs=4, space="PSUM") as psV,
            ):
                for b in range(BPC):
                    # ---- load x_b as [P, KT, S]; keep exact f32 for the
                    # residual plus a DVE-rounded f32r copy for matmuls.
                    x_t = xp.tile([P, KT, HW], F32, tag="x", name=f"x{b}")
                    nc.sync.dma_start(
                        out=x_t, in_=x_d[b].rearrange("(kt p) s -> p kt s", p=P)
                    )
                    x_r = xp.tile([P, KT, HW], MMDT, tag="xr", name=f"xr{b}")
                    nc.vector.tensor_copy(
                        out=x_r.rearrange("p kt s -> p (kt s)"),
                        in_=x_t.rearrange("p kt s -> p (kt s)"),
                    )

                    # ---- phase 1: q^T/k^T per head ----
                    # qkT[P, {q,k}, pair, S]: partitions 0-63 head 2a,
                    # 64-127 head 2a+1.
                    qkT = qktp.tile([P, 2, NPAIR, HW], MMDT, tag="qkT",
                                    name=f"qkT{b}")
                    for h in range(NH):
                        a, m = h // 2, h % 2
                        ps = psS.tile([P, 1024], F32, tag="mm", name=f"ps_qk{h}")
                        for sc in range(SC):
                            for kt in range(KT):
                                nc.tensor.matmul(
                                    ps[:, sc * 512 : (sc + 1) * 512],
                                    lhsT=wqkv_r[:, kt, h, 0 : 2 * DK],
                                    rhs=x_r[:, kt, sc * 512 : (sc + 1) * 512],
                                    start=(kt == 0),
                                    stop=(kt == KT - 1),
                                )
                        half = slice(m * DK, (m + 1) * DK)
                        for qk in range(2):  # 0=q (psum 0:64), 1=k (psum 64:128)
                            src = ps[qk * DK : (qk + 1) * DK, :]
                            if with_bias:
                                bsb = bq_sb if qk == 0 else bk_sb
                                nc.vector.tensor_scalar(
                                    out=qkT[half, qk, a, :],
                                    in0=src,
                                    scalar1=bsb[half, a : a + 1],
                                    scalar2=None,
                                    op0=mybir.AluOpType.add,
                                )
                            else:
                                nc.vector.tensor_copy(
                                    out=qkT[half, qk, a, :], in_=src
                                )

                    # ---- phase 1: v natural [P(s), st, head, 65], ones col ----
                    v_sb = vp.tile([P, ST, NH, DK + 1], pv_dt, tag="v",
                                   name=f"v{b}")
                    nc.vector.memset(
                        v_sb.rearrange("p st h t -> p (st h) t")[:, :, DK:], 1.0
                    )
                    for mt2 in range(ST // 2):
                        ps = psS.tile([P, 1024], F32, tag="mm", name=f"ps_v{mt2}")
                        for half_i in range(2):
                            mt = 2 * mt2 + half_i
                            for kt in range(KT):
                                nc.tensor.matmul(
                                    ps[:, half_i * 512 : (half_i + 1) * 512],
                                    lhsT=x_r[:, kt, mt * P : (mt + 1) * P],
                                    rhs=wv_sb[:, kt, :],
                                    start=(kt == 0),
                                    stop=(kt == KT - 1) if not with_bias else False,
                                )
                            if with_bias:
                                # += ones^T @ b_v (adds b_v to every row)
                                nc.tensor.matmul(
                                    ps[:, half_i * 512 : (half_i + 1) * 512],
                                    lhsT=ones_sb,
                                    rhs=bv_sb,
                                    start=False,
                                    stop=True,
                                )
                        for half_i in range(2):
                            mt = 2 * mt2 + half_i
                            nc.vector.tensor_copy(
                                out=v_sb[:, mt, :, 0:DK],
                                in_=ps[:, half_i * 512 : (half_i + 1) * 512]
                                .rearrange("p (h t) -> p h t", h=NH),
                            )

                    # ---- phase 2: attention per head pair ----
                    oT = otp.tile([P, NPAIR, HW], MMDT, tag="oT", name=f"oT{b}")
                    for a in range(NPAIR):
                        pts = [
                            ptp.tile([P, ST, HW], pv_dt, tag="pt",
                                     name=f"pt{a}_{m}")
                            for m in range(2)
                        ]
                        # scores^T + exp; pair co-runs on PE row halves
                        for jt in range(ST):
                            pss = [
                                psS.tile([P, 1024], F32, tag="mm",
                                         name=f"ps_s{m}")
                                for m in range(2)
                            ]
                            for sc in range(SC):
                                for m in range(2):
                                    lo, hi = m * DK, (m + 1) * DK
                                    nc.tensor.matmul(
                                        pss[m][:, sc * 512 : (sc + 1) * 512],
                                        lhsT=qkT[lo:hi, 1, a,
                                                 jt * P : (jt + 1) * P],
                                        rhs=qkT[lo:hi, 0, a,
                                                sc * 512 : (sc + 1) * 512],
                                        start=True,
                                        stop=True,
                                    )
                            for m in range(2):
                                nc.scalar.activation(
                                    out=pts[m][:, jt, :],
                                    in_=pss[m],
                                    func=mybir.ActivationFunctionType.Exp,
                                    scale=SCALE,
                                )

                        # P@V + rowsum, normalize -> oT
                        for m in range(2):
                            h = 2 * a + m
                            for sc in range(SC):
                                pv = psV.tile([DK + 1, 512], F32, tag="pv",
                                              name="pv")
                                for jt in range(ST):
                                    nc.tensor.matmul(
                                        pv,
                                        lhsT=v_sb[:, jt, h, :],
                                        rhs=pts[m][:, jt,
                                                   sc * 512 : (sc + 1) * 512],
                                        start=(jt == 0),
                                        stop=(jt == ST - 1),
                                    )
                                # 1/rowsum: fast approx (rowsum >= ~1, no edge
                                # cases; 51 ULP is far below our error budget)
                                # then broadcast over 64 partitions on GpSimd.
                                # (custom-DVE ops misread PSUM/offset inputs;
                                # bounce the rowsum to SBUF partition 0 first)
                                rs = rcp.tile([1, 512], F32, tag="rs", name="rs")
                                nc.vector.tensor_copy(
                                    out=rs, in_=pv[DK : DK + 1, :]
                                )
                                rc = rcp.tile([1, 512], F32, tag="rc", name="rc")
                                nc.vector.reciprocal_approx_fast(out=rc, in_=rs)
                                rcb = rcbp.tile([DK, 512], F32, tag="rcb",
                                                name="rcb")
                                nc.gpsimd.partition_broadcast(rcb, rc)
                                osl = oT[m * DK : (m + 1) * DK, a,
                                         sc * 512 : (sc + 1) * 512]
                                nc.vector.tensor_tensor(
                                    out=osl, in0=pv[0:DK, :], in1=rcb,
                                    op=mybir.AluOpType.mult,
                                )

                    # ---- phase 3: proj + bias + residual ----
                    for a in range(KT):
                        ps = psS.tile([P, 1024], F32, tag="mm", name=f"ps_p{a}")
                        for sc in range(SC):
                            for kt in range(KT):
                                nc.tensor.matmul(
                                    ps[:, sc * 512 : (sc + 1) * 512],
                                    lhsT=wproj_r[:, kt, a * P : (a + 1) * P],
                                    rhs=oT[:, kt, sc * 512 : (sc + 1) * 512],
                                    start=(kt == 0),
                                    stop=(kt == KT - 1),
                                )
                        yt = stagep.tile([P, 1024], F32, tag="y", name=f"yt{a}")
                        if with_bias:
                            nc.vector.tensor_scalar(
                                out=yt, in0=ps, scalar1=bp_sb[:, a : a + 1],
                                scalar2=None, op0=mybir.AluOpType.add,
                            )
                            nc.vector.tensor_tensor(
                                out=yt, in0=yt, in1=x_t[:, a, :],
                                op=mybir.AluOpType.add,
                            )
                        else:
                            nc.vector.tensor_tensor(
                                out=yt, in0=ps, in1=x_t[:, a, :],
                                op=mybir.AluOpType.add,
                            )
                        nc.sync.dma_start(
                            out=out_d[b].rearrange("(kt p) s -> p kt s", p=P)
                            [:, a, :],
                            in_=yt,
                        )

    nc.finalize()
    return nc


_CACHE = {}


def _get_program(with_bias: bool, mm_dtype: str = MM_DTYPE):
    key = (with_bias, mm_dtype)
    if key not in _CACHE:
        _CACHE[key] = build_program(with_bias, mm_dtype)
    return _CACHE[key]


def kernel(x, w_qkv, b_qkv, w_proj, b_proj):
    x = np.ascontiguousarray(np.asarray(x, dtype=np.float32)).reshape(B, C, HW)
    w_qkv = np.ascontiguousarray(np.asarray(w_qkv, dtype=np.float32))
    b_qkv = np.ascontiguousarray(np.asarray(b_qkv, dtype=np.float32))
    w_proj = np.ascontiguousarray(np.asarray(w_proj, dtype=np.float32))
    b_proj = np.ascontiguousarray(np.asarray(b_proj, dtype=np.float32))

    with_bias = bool(np.any(b_qkv) or np.any(b_proj))
    nc = _get_program(with_bias, MM_DTYPE)

    in_maps = []
    for i in range(NCORES):
        m = {
            "x": x[i * BPC : (i + 1) * BPC],
            "w_qkv": w_qkv,
            "w_proj": w_proj,
        }
        if with_bias:
            m["b_qkv"] = b_qkv
            m["b_proj"] = b_proj
        in_maps.append(m)

    from concourse.bass_utils import run_bass_kernel_spmd

    res = run_bass_kernel_spmd(nc, in_maps, core_ids=list(range(NCORES)))
    out = np.concatenate([r["out"] for r in res.results], axis=0)
    return out.reshape(B, C, 32, 32)
